# revision 1
# baseline (speedup 1.0000x reference)
"""Trainium2 Bass kernel for nn_Attention_1056561955116 (sparse chunk attention).

Contract: kernel(**inputs) takes FULL unsharded numpy inputs (as produced by
the problem's setup_inputs) and returns the FULL [2, 2048, 1024] f32 output.

Sharding: 8 NeuronCores = 2 batches x 4 head-groups (4 heads / 256 inner dims
each). Per core: QKV projections (bf16, f32 accum) pipelined per 512-token
block with qk-RMS-norm + rotary on vector/scalar engines, block-sparse
attention (41 of 256 chunk pairs) with two heads packed per PE pass,
softmax denominator via the ones-column trick + fast reciprocal; per
512-token block the attention output (head-sharded) is AllToAll-resharded
to token-sharded, each core then runs the full output projection for its
128-token chunk of the block. Host concatenates the per-core row blocks.
"""

import os
import sys

import numpy as np

for _p in ("/opt/trn_rl_repo", "/root/.axon_site/_ro/trn_rl_repo"):
    if os.path.isdir(_p) and _p not in sys.path:
        sys.path.append(_p)

import concourse.bass as bass  # noqa: E402,F401
import concourse.mybir as mybir  # noqa: E402
import concourse.tile as tile  # noqa: E402
from concourse import bacc  # noqa: E402
from concourse.bass_utils import run_bass_kernel_spmd  # noqa: E402

try:
    import ml_dtypes

    BF16_NP = ml_dtypes.bfloat16
except ImportError:  # pragma: no cover
    BF16_NP = np.float32

F32 = mybir.dt.float32
F32R = mybir.dt.float32r
BF16 = mybir.dt.bfloat16
I32 = mybir.dt.int32
AF = mybir.ActivationFunctionType
OP = mybir.AluOpType

B = 2
S = 2048
D = 1024
HEADS = 16
HD = 64
CHUNK = 128
N_CORES = 8
HG = 4          # heads per core
E = HG * HD     # 256 inner dims per core
NCH = S // CHUNK  # 16 chunks
GROUPS = [[0, 1, 2, 3], [4, 5, 6, 7]]
EPS = 1e-6
SCALE = 1.0 / float(HD) ** 0.5


def _build_chunk_lists():
    n_cache = S // (2 * CHUNK)  # 8
    max_lookback = 5
    K = []  # K[j] = key chunks visible to query chunk j, self last
    for j in range(NCH):
        ks = []
        rel = j - n_cache
        if rel >= 0:
            ks = [c for c in range(n_cache)
                  if c < rel and c >= rel - max_lookback]
        ks.append(j)
        K.append(ks)
    return K


K_J = _build_chunk_lists()
assert sum(len(k) for k in K_J) == 41

# Attention plan per query bank-block jb: groups of <= 4 (c, j) chunk pairs
# (512 logit columns per head = 1 psum bank per head). Entries within a
# group are c-major merged j-runs with uniform start/stop flags; c-ascending
# order guarantees each j's start entry (first cache chunk) executes first
# and its stop entry (self chunk) executes last.


def _build_attn_plan():
    plans = []  # plans[jb] = list of (entries, ncol); entry=(c,j0,jn,col,st,sp)
    for jb in range(4):
        js = range(4 * jb, 4 * jb + 4)
        cs = sorted({c for j in js for c in K_J[j]})
        seq = [(c, j) for c in cs for j in js if c in K_J[j]]
        groups = [seq[i:i + 4] for i in range(0, len(seq), 4)]
        gplans = []
        for grp in groups:
            entries = []
            col = 0
            for (c, j) in grp:
                st = (c == K_J[j][0])
                sp = (c == j)
                if (entries and entries[-1][0] == c
                        and entries[-1][1] + entries[-1][2] == j
                        and entries[-1][4] == st and entries[-1][5] == sp):
                    e = entries[-1]
                    entries[-1] = (e[0], e[1], e[2] + 1, e[3], e[4], e[5])
                else:
                    entries.append((c, j, 1, col, st, sp))
                col += CHUNK
            gplans.append((entries, col))
        plans.append(gplans)
    return plans


ATTN_PLAN = _build_attn_plan()


def _dev2orig():
    # rotation pair i -> device lanes (32*(i//16) + i%16, +16): the rope
    # swap partner is lane ^ 16 inside each 32-lane group.
    d2o = np.zeros(HD, dtype=np.int64)
    for a in range(HD):
        q32, r0 = divmod(a, 32)
        o, r = divmod(r0, 16)
        d2o[a] = 2 * (16 * q32 + r) + o
    return d2o


DEV2ORIG = _dev2orig()
SWAP = DEV2ORIG[np.arange(HD) ^ 16]

_PROGRAM_CACHE = {}


class _Bacc(bacc.Bacc):
    def insert_act_table_loads(self):
        from concourse.hw_specs import get_activation_tables

        has_activation = any(
            isinstance(i, mybir.InstActivation)
            for b in self.main_func.blocks
            for i in b.instructions
        )
        if not has_activation:
            return
        A = mybir.ActivationFunctionType
        keep_only_in = "natural_log_exp_and_others"
        steer = {A.Exp, A.Ln}
        tables = []
        for name, fns in get_activation_tables(self.m.arch).items():
            if name != keep_only_in:
                fns = {f for f in fns if f not in steer}
            tables.append((name, fns))
        import bass_rust as _bass_rust
        _bass_rust.insert_act_table_loads(self, tables)


def build_program(fold_w=False):
    key = "prog"
    if key in _PROGRAM_CACHE:
        return _PROGRAM_CACHE[key]

    nc = _Bacc("TRN2", target_bir_lowering=False, debug=False,
               num_devices=N_CORES)

    xT = nc.dram_tensor("xT", [D, S], BF16, kind="ExternalInput")
    wqT = nc.dram_tensor("wqT", [D, E], BF16, kind="ExternalInput")
    wkT = nc.dram_tensor("wkT", [D, E], BF16, kind="ExternalInput")
    wvT = nc.dram_tensor("wvT", [D, E], BF16, kind="ExternalInput")
    woT = nc.dram_tensor("woT", [D, D], BF16, kind="ExternalInput")
    bqd = nc.dram_tensor("bq", [E], F32, kind="ExternalInput")
    bkd = nc.dram_tensor("bk", [E], F32, kind="ExternalInput")
    bod = nc.dram_tensor("bo_eff", [128, D], BF16, kind="ExternalInput")
    cosqd = nc.dram_tensor("cosq", [128, S], BF16, kind="ExternalInput")
    sinqd = nc.dram_tensor("sinq", [128, S], BF16, kind="ExternalInput")
    coskd = nc.dram_tensor("cosk", [128, S], BF16, kind="ExternalInput")
    sinkd = nc.dram_tensor("sink", [128, S], BF16, kind="ExternalInput")
    ind4d = nc.dram_tensor("ind4", [128, 4, 8], BF16, kind="ExternalInput")
    sel3d = nc.dram_tensor("sel3", [8, 512], BF16, kind="ExternalInput")
    bmaskd = nc.dram_tensor("bmask", [128, 16 * 128], BF16, kind="ExternalInput")

    out_ext = nc.dram_tensor("out", [4 * 128, D], BF16, kind="ExternalOutput")
    kdbg = bool(os.environ.get("KDBG"))
    if kdbg:
        dbg = {
            "dbg_qh0": nc.dram_tensor("dbg_qh0", [128, S], BF16, kind="ExternalOutput"),
            "dbg_kh0": nc.dram_tensor("dbg_kh0", [128, S], BF16, kind="ExternalOutput"),
            "dbg_rallb": nc.dram_tensor("dbg_rallb", [8, S], BF16, kind="ExternalOutput"),
            "dbg_pch00": nc.dram_tensor("dbg_pch00", [128, 1024], BF16, kind="ExternalOutput"),
            "dbg_usl00": nc.dram_tensor("dbg_usl00", [1, 512], F32, kind="ExternalOutput"),
            "dbg_oun00": nc.dram_tensor("dbg_oun00", [HD, 512], BF16, kind="ExternalOutput"),
            "dbg_ofin0": nc.dram_tensor("dbg_ofin0", [128, S], BF16, kind="ExternalOutput"),
            "dbg_of80": nc.dram_tensor("dbg_of80", [128, 16 * 128], BF16, kind="ExternalOutput"),
            "dbg_ofa0": nc.dram_tensor("dbg_ofa0", [128, 8 * 128], BF16, kind="ExternalOutput"),
        }
    # 8-core AllToAll (4-core mesh A2A unsupported): every core sends its
    # chunk-(p%4) o-slices to peer p; cross-batch slots arrive as junk and
    # are zeroed by the per-core bmask before the slot-pair add. Two jb
    # blocks ride per collective — fewer sync points, less skew cost.
    a2a_in = [nc.dram_tensor(f"a2a_in{pj}", [8, 2, 2, 128, 128], BF16)
              for pj in range(2)]
    a2a_out = [nc.dram_tensor(f"a2a_out{pj}", [8, 2, 2, 128, 128], BF16)
               for pj in range(2)]
    warm_in = nc.dram_tensor("warm_in", [8, 32], BF16)
    warm_out = nc.dram_tensor("warm_out", [8, 32], BF16)

    with tile.TileContext(nc) as tc:
        with tc.tile_pool(name="persist", bufs=1) as pp, \
             tc.tile_pool(name="tmp", bufs=1) as tp:

            x_sb = pp.tile([128, 8, S], BF16, name="x_sb")
            wq_sb = pp.tile([128, 8, E], BF16, name="wq_sb")
            wk_sb = pp.tile([128, 8, E], BF16, name="wk_sb")
            wv_sb = pp.tile([128, 8, E], BF16, name="wv_sb")
            wo_sb = pp.tile([128, 8, D], BF16, name="wo_sb")
            bq_sb = pp.tile([128, 2], F32, name="bq_sb")
            bk_sb = pp.tile([128, 2], F32, name="bk_sb")
            bo_sb = pp.tile([128, D], BF16, name="bo_sb")
            ind4_sb = pp.tile([128, 4, 8], BF16, name="ind4_sb")
            sel3_sb = pp.tile([8, 512], BF16, name="sel3_sb")
            bm_sb = pp.tile([128, 16 * 128], BF16, name="bm_sb")
            on64f_sb = pp.tile([128, HD], F32, name="on64f_sb")
            on64_sb = pp.tile([128, HD], F32R, name="on64_sb")
            eps_sb = pp.tile([128, 1], F32, name="eps_sb")

            qt_sb = [pp.tile([128, S], BF16, name=f"qt{t}") for t in range(2)]
            kt_sb = [pp.tile([128, S], BF16, name=f"kt{t}") for t in range(2)]
            qh_sb = [pp.tile([128, S], BF16, name=f"qh{t}") for t in range(2)]
            kh_sb = [pp.tile([128, S], BF16, name=f"kh{t}") for t in range(2)]
            v_sb = pp.tile([128, NCH, HG * (HD + 1)], BF16, name="v_sb")
            o_fin = [pp.tile([128, S], BF16, name=f"ofin{t}") for t in range(2)]
            rallb = pp.tile([8, S], BF16, name="rallb")
            cosq = pp.tile([128, S], BF16, name="cosq")
            sinq = pp.tile([128, S], BF16, name="sinq")
            cosk = pp.tile([128, S], BF16, name="cosk")
            sink = pp.tile([128, S], BF16, name="sink")

            # ---- input DMAs: x m0 + q/k weights first, wo/bias last ----
            for dc in range(8):
                nc.sync.dma_start(x_sb[:, dc, 0:512], xT[128 * dc:128 * (dc + 1), 0:512])
            nc.sync.dma_start(wq_sb[:], wqT.ap().rearrange("(c p) e -> p c e", p=128))
            nc.sync.dma_start(wk_sb[:], wkT.ap().rearrange("(c p) e -> p c e", p=128))
            nc.sync.dma_start(bq_sb[:], bqd.ap().rearrange("(t p) -> p t", p=128))
            nc.sync.dma_start(bk_sb[:], bkd.ap().rearrange("(t p) -> p t", p=128))
            nc.sync.dma_start(ind4_sb[:], ind4d[:])
            nc.sync.dma_start(sel3_sb[:], sel3d[:])
            nc.sync.dma_start(cosq[:], cosqd[:])
            nc.sync.dma_start(sinq[:], sinqd[:])
            nc.sync.dma_start(cosk[:], coskd[:])
            nc.sync.dma_start(sink[:], sinkd[:])
            for m in range(1, 4):
                for dc in range(8):
                    nc.sync.dma_start(
                        x_sb[:, dc, 512 * m:512 * (m + 1)],
                        xT[128 * dc:128 * (dc + 1), 512 * m:512 * (m + 1)])
            nc.sync.dma_start(wv_sb[:], wvT.ap().rearrange("(c p) e -> p c e", p=128))
            nc.sync.dma_start(wo_sb[:], woT.ap().rearrange("(c p) d -> p c d", p=128))
            nc.sync.dma_start(bo_sb[:], bod[:])
            nc.sync.dma_start(bm_sb[:], bmaskd[:])

            nc.vector.memset(eps_sb[:], EPS)
            nc.vector.memset(on64f_sb[:], 1.0)
            nc.vector.tensor_copy(on64_sb[:], on64f_sb[:])
            # tiny dummy collective: absorbs the first-op stream warm-up +
            # inter-core skew so the real AllToAlls run at full rate
            nc.gpsimd.collective_compute(
                "AllToAll", OP.bypass, replica_groups=[list(range(8))],
                ins=[warm_in[:]], outs=[warm_out[:]])
            nc.vector.memset(
                v_sb[:].rearrange("p t (h x) -> p t h x", x=HD + 1)[:, :, :, HD:],
                1.0)

            # tiles: i -> (weights, bias, pre-rope dst, rope tables, rope dst)
            tiles = [
                (wq_sb, bq_sb, qt_sb[0], 0, cosq, sinq, qh_sb[0]),
                (wq_sb, bq_sb, qt_sb[1], 1, cosq, sinq, qh_sb[1]),
                (wk_sb, bk_sb, kt_sb[0], 0, cosk, sink, kh_sb[0]),
                (wk_sb, bk_sb, kt_sb[1], 1, cosk, sink, kh_sb[1]),
            ]

            # ---------------- m-loop: projections + rms + rope ----------------
            with tc.tile_pool(name="p1", bufs=1, space="PSUM") as p1:
                for m in range(4):
                    msl = slice(512 * m, 512 * (m + 1))
                    # q0 q1 k0 k1 projections; each tile's pssq matmul is
                    # staggered one tile later so the PE never waits on the
                    # scalar copy + vector square feeding it
                    sqs = []
                    for i, (w_sb, b_sb, dst, t, ct, st_t, rdst) in enumerate(tiles):
                        ps = p1.tile([128, 512], F32, tag="proj", bufs=2)
                        for dc in range(8):
                            nc.tensor.matmul(
                                ps[:], w_sb[:, dc, 128 * t:128 * (t + 1)],
                                x_sb[:, dc, msl],
                                start=(dc == 0), stop=(dc == 7))
                        nc.scalar.activation(dst[:, msl], ps[:], AF.Identity,
                                             bias=b_sb[:, t:t + 1])
                        sq = tp.tile([128, 512], BF16, tag="sq", bufs=3)
                        nc.vector.tensor_tensor(sq[:], dst[:, msl], dst[:, msl],
                                                OP.mult)
                        sqs.append(sq)
                        if i >= 1:
                            if i == 1:
                                pssq = p1.tile([128, 512], F32, tag="pssq",
                                               bufs=2)
                            nc.tensor.matmul(pssq[0:8, :], ind4_sb[:, i - 1, :],
                                             sqs[i - 1][:],
                                             start=(i == 1), stop=False)
                    # v projection for this m-block's 4 chunks; last pssq
                    # rides after the first v chunk
                    for k, tn in enumerate(range(4 * m, 4 * m + 4)):
                        vps = p1.tile([128, 256], F32, tag="vps", bufs=2)
                        for dc in range(8):
                            nc.tensor.matmul(
                                vps[:], x_sb[:, dc, 128 * tn:128 * (tn + 1)],
                                wv_sb[:, dc, :], start=(dc == 0), stop=(dc == 7))
                        vdst = v_sb[:, tn].rearrange("p (h x) -> p h x",
                                                     x=HD + 1)[:, :, :HD]
                        nc.scalar.activation(
                            vdst, vps[:].rearrange("p (h d) -> p h d", d=HD),
                            AF.Copy)
                        if k == 0:
                            nc.tensor.matmul(pssq[0:8, :], ind4_sb[:, 3, :],
                                             sqs[3][:], start=False, stop=True)
                    # rsqrt factors for this m-block (rows 2*i + [0,1])
                    rall_m = tp.tile([8, 512], F32, tag="rall", bufs=2)
                    nc.scalar.activation(rall_m[:], pssq[0:8, :], AF.Ln,
                                         scale=1.0 / HD, bias=eps_sb[0:8, 0:1])
                    nc.scalar.activation(rallb[:, msl], rall_m[:], AF.Exp,
                                         scale=-0.5)
                    # rope + rms scale, on vector engine; rb broadcast on PE
                    for i, (w_sb, b_sb, src, t, ct, st_t, rdst) in enumerate(tiles):
                        sh = tp.tile([128, 512], BF16, tag="sh", bufs=2)
                        nc.vector.stream_shuffle(sh[:].bitcast(I32),
                                                 src[:, msl].bitcast(I32),
                                                 [l ^ 16 for l in range(32)])
                        t1 = tp.tile([128, 512], BF16, tag="t1", bufs=2)
                        nc.vector.tensor_tensor(t1[:], src[:, msl], ct[:, msl],
                                                OP.mult)
                        t2 = tp.tile([128, 512], BF16, tag="t2", bufs=2)
                        nc.vector.tensor_tensor(t2[:], sh[:], st_t[:, msl],
                                                OP.mult)
                        nc.vector.tensor_tensor(t1[:], t1[:], t2[:], OP.add)
                        rb = p1.tile([128, 512], F32, tag="rb", bufs=2)
                        nc.tensor.matmul(rb[:], sel3_sb[:, 128 * i:128 * (i + 1)],
                                         rallb[:, msl], start=True, stop=True)
                        nc.vector.tensor_tensor(rdst[:, msl], t1[:], rb[:],
                                                OP.mult)

            if kdbg:
                nc.sync.dma_start(dbg["dbg_qh0"][:], qh_sb[0][:])
                nc.sync.dma_start(dbg["dbg_kh0"][:], kh_sb[0][:])
                nc.sync.dma_start(dbg["dbg_rallb"][:], rallb[:])

            # ---------------- attention + A2A + out-proj ----------------
            # Emission order pipelines: attn jb0, attn jb1, oproj jb0,
            # attn jb2, oproj jb1, attn jb3, oproj jb2, oproj jb3 — the
            # AllToAll latency of block jb hides behind attention jb+1.
            def emit_attn(jb, p2):
                n_ent = sum(len(e) for e, _ in ATTN_PLAN[jb])
                for th in range(2):
                    pvs = []
                    for h in range(2):
                        pv = p2.tile([HD + 1, 512], F32, tag="pv", bufs=2)
                        pvs.append(pv)
                    ei = 0
                    for (entries, ncol) in ATTN_PLAN[jb]:
                        lt = p2.tile([128, 1024], F32, tag="lt", bufs=2)
                        for (c, j0, jn, col, st, sp) in entries:
                            csl = slice(128 * c, 128 * (c + 1))
                            jsl = slice(128 * j0, 128 * (j0 + jn))
                            nc.tensor.matmul(
                                lt[:, col:col + 128 * jn],
                                kh_sb[th][0:64, csl], qh_sb[th][0:64, jsl],
                                start=True, stop=True)
                            nc.tensor.matmul(
                                lt[:, 512 + col:512 + col + 128 * jn],
                                kh_sb[th][64:128, csl], qh_sb[th][64:128, jsl],
                                start=True, stop=True)
                        pch = tp.tile([128, 1024], BF16, tag="pch", bufs=3)
                        if ncol == 512:
                            nc.scalar.activation(pch[:, 0:1024], lt[:, 0:1024],
                                                 AF.Exp, scale=SCALE)
                        else:
                            for h in range(2):
                                nc.scalar.activation(
                                    pch[:, 512 * h:512 * h + ncol],
                                    lt[:, 512 * h:512 * h + ncol],
                                    AF.Exp, scale=SCALE)
                        if kdbg and jb == 0 and th == 0 and ei == 0:
                            nc.sync.dma_start(dbg["dbg_pch00"][:], pch[:])
                        for (c, j0, jn, col, st, sp) in entries:
                            jj = j0 - 4 * jb
                            for h in range(2):
                                vsl = v_sb[:, c].rearrange(
                                    "p (h x) -> p h x", x=HD + 1)[:, 2 * th + h, :]
                                nc.tensor.matmul(
                                    pvs[h][:, 128 * jj:128 * (jj + jn)], vsl,
                                    pch[:, 512 * h + col:512 * h + col + 128 * jn],
                                    start=(ei == 0), stop=(ei == n_ent - 1))
                            ei += 1
                    # normalize: o = pv[:64] * exp(-ln(den)); ln(den) row is
                    # broadcast to 64 lanes with a full-rate f32r matmul
                    for h in range(2):
                        pv = pvs[h]
                        usl = tp.tile([HD + 1, 512], F32, tag="usl", bufs=2)
                        nc.scalar.activation(usl[HD:HD + 1, :],
                                             pv[HD:HD + 1, :], AF.Ln)
                        uslr = tp.tile([HD + 1, 512], F32R, tag="uslr", bufs=2)
                        nc.vector.tensor_copy(uslr[HD:HD + 1, :],
                                              usl[HD:HD + 1, :])
                        rdb = p2.tile([128, 512], F32, tag="ops", bufs=2)
                        nc.tensor.matmul(
                            rdb[0:HD, :], on64_sb[HD:HD + 1, 0:HD],
                            uslr[HD:HD + 1, :],
                            start=True, stop=True)
                        rdb_sb = tp.tile([HD, 512], BF16, tag="rdb_sb", bufs=2)
                        nc.scalar.activation(rdb_sb[:], rdb[0:HD, :], AF.Exp,
                                             scale=-1.0)
                        nc.vector.tensor_tensor(
                            o_fin[th][64 * h:64 * (h + 1),
                                      512 * jb:512 * (jb + 1)],
                            pv[0:HD, :], rdb_sb[:], OP.mult)
                        if kdbg and jb == 0 and th == 0 and h == 0:
                            nc.sync.dma_start(dbg["dbg_usl00"][:],
                                              usl[HD:HD + 1, :])
                            nc.sync.dma_start(dbg["dbg_oun00"][:], rdb_sb[:])
            def emit_a2a(pj):
                # ship o^T [E, tok] slices: peer p gets my chunk p%4 of both
                # jb blocks in this pair
                for p in range(8):
                    pc = p % 4
                    for j2 in range(2):
                        jb = 2 * pj + j2
                        for th in range(2):
                            nc.sync.dma_start(
                                a2a_in[pj][p, j2, th],
                                o_fin[th][:, 512 * jb + 128 * pc:
                                          512 * jb + 128 * (pc + 1)])
                if kdbg and pj == 0:
                    nc.sync.dma_start(dbg["dbg_ofin0"][:, 0:512],
                                      o_fin[0][:, 0:512])
                nc.gpsimd.collective_compute(
                    "AllToAll", OP.bypass, replica_groups=[list(range(8))],
                    ins=[a2a_in[pj][:]], outs=[a2a_out[pj][:]])

            def emit_of(pj):
                of2 = tp.tile([128, 2, 16, 128], BF16, tag="of2", bufs=2)
                # on the gpsimd queue: FIFO-ordered behind the AllToAll, so
                # the read cannot race the collective's DRAM writes
                for j2 in range(2):
                    for t1 in range(2):
                        nc.gpsimd.dma_start(
                            of2[:, j2].rearrange("p (q t) k -> p q t k",
                                                 t=2)[:, :, t1],
                            a2a_out[pj][:, j2, t1].rearrange("q l k -> l q k"))
                ofs = []
                for j2 in range(2):
                    ofa = tp.tile([128, 8, 128], BF16, tag="ofa", bufs=2)
                    ofb = tp.tile([128, 8, 128], BF16, tag="ofb", bufs=2)
                    o8f = of2[:, j2].rearrange("p a b -> p (a b)")
                    nc.vector.tensor_tensor(
                        ofa[:].rearrange("p a b -> p (a b)"),
                        o8f[:, 0:1024], bm_sb[:, 0:1024], OP.mult)
                    nc.vector.tensor_tensor(
                        ofb[:].rearrange("p a b -> p (a b)"),
                        o8f[:, 1024:2048], bm_sb[:, 1024:2048], OP.mult)
                    nc.vector.tensor_tensor(
                        ofa[:].rearrange("p a b -> p (a b)"),
                        ofa[:].rearrange("p a b -> p (a b)"),
                        ofb[:].rearrange("p a b -> p (a b)"), OP.add)
                    ofs.append(ofa)
                if kdbg and pj == 0:
                    nc.sync.dma_start(
                        dbg["dbg_of80"][:],
                        of2[:, 0].rearrange("p a b -> p (a b)"))
                    nc.sync.dma_start(
                        dbg["dbg_ofa0"][:],
                        ofs[0][:].rearrange("p a b -> p (a b)"))
                return ofs

            def emit_oproj(jb, ofa, p2):
                for dh in range(2):
                    ops = p2.tile([128, 512], F32, tag="ops", bufs=2)
                    for ec in range(8):
                        nc.tensor.matmul(
                            ops[:], ofa[:, ec, :],
                            wo_sb[:, ec, 512 * dh:512 * (dh + 1)],
                            start=(ec == 0), stop=(ec == 7))
                    st = tp.tile([128, 512], BF16, tag="st", bufs=2)
                    nc.vector.tensor_tensor(
                        st[:], ops[:], bo_sb[:, 512 * dh:512 * (dh + 1)], OP.add)
                    nc.sync.dma_start(
                        out_ext[128 * jb:128 * (jb + 1),
                                512 * dh:512 * (dh + 1)], st[:])

            # all A2A triggers go on the gpsimd queue before any of2
            # consumer DMA — the collectives pipeline on the CC stream while
            # the PE finishes attention; out-projections drain at the end.
            with tc.tile_pool(name="p2", bufs=1, space="PSUM") as p2:
                emit_attn(0, p2)
                emit_attn(1, p2)
                emit_a2a(0)
                emit_attn(2, p2)
                emit_attn(3, p2)
                emit_a2a(1)
                ofs0 = emit_of(0)
                emit_oproj(0, ofs0[0], p2)
                emit_oproj(1, ofs0[1], p2)
                ofs1 = emit_of(1)
                emit_oproj(2, ofs1[0], p2)
                emit_oproj(3, ofs1[1], p2)

    nc.compile()
    _PROGRAM_CACHE[key] = nc
    return nc


def prep_core_inputs(x, mask, freqs, Wq, bq, Wk, bk, Wv, bv, Wo, bo, qw, kw):
    qw = np.asarray(qw, np.float64)
    kw = np.asarray(kw, np.float64)

    perm = (np.arange(HEADS)[:, None] * HD + DEV2ORIG[None, :]).reshape(-1)
    Wq_p = np.asarray(Wq, np.float32)[perm]
    Wk_p = np.asarray(Wk, np.float32)[perm]
    bq_p = np.asarray(bq, np.float32)[perm]
    bk_p = np.asarray(bk, np.float32)[perm]

    fr = np.asarray(freqs, np.float64)[:, DEV2ORIG].T  # [64, S] dev-lane order
    sign = np.where((np.arange(HD) % 32) < 16, -1.0, 1.0)
    cos_d = np.cos(fr)
    sin_d = np.sin(fr) * sign[:, None]
    cosq1 = cos_d * qw[DEV2ORIG][:, None]
    sinq1 = sin_d * qw[SWAP][:, None]
    cosk1 = cos_d * kw[DEV2ORIG][:, None]
    sink1 = sin_d * kw[SWAP][:, None]

    def dup(a):
        return np.concatenate([a, a], axis=0).astype(BF16_NP)

    ind4 = np.zeros((128, 4, 8), np.float32)
    for i in range(4):
        ind4[0:64, i, 2 * i] = 1.0
        ind4[64:128, i, 2 * i + 1] = 1.0
    sel3 = np.zeros((8, 512), np.float32)
    for i in range(4):
        for b2 in range(2):
            sel3[2 * i + b2, 128 * i + 64 * b2:128 * i + 64 * (b2 + 1)] = 1.0

    x = np.asarray(x, np.float32)
    Wo = np.asarray(Wo, np.float32)
    Wv = np.asarray(Wv, np.float32)
    bv = np.asarray(bv, np.float32)
    bo = np.asarray(bo, np.float32)
    woT = np.ascontiguousarray(Wo.T)  # [E_full=1024, D]
    bo_eff = bv @ woT + bo            # [D]

    in_maps = []
    for core in range(N_CORES):
        b, g = divmod(core, 4)
        esl = slice(E * g, E * (g + 1))
        # receive mask: slot (src, th) columns are 1 iff src is in my batch
        bmask = np.zeros((128, 16, 128), np.float32)
        for src in range(8):
            if src // 4 == b:
                for th in range(2):
                    bmask[:, 2 * src + th, :] = 1.0
        m = {
            "xT": np.ascontiguousarray(x[b].T).astype(BF16_NP),
            "wqT": np.ascontiguousarray(Wq_p[esl].T).astype(BF16_NP),
            "wkT": np.ascontiguousarray(Wk_p[esl].T).astype(BF16_NP),
            "wvT": np.ascontiguousarray(Wv[esl].T).astype(BF16_NP),
            "woT": woT.astype(BF16_NP),
            "bq": bq_p[esl].copy(),
            "bk": bk_p[esl].copy(),
            "bo_eff": np.broadcast_to(bo_eff.astype(BF16_NP), (128, D)).copy(),
            "cosq": dup(cosq1),
            "sinq": dup(sinq1),
            "cosk": dup(cosk1),
            "sink": dup(sink1),
            "ind4": ind4.astype(BF16_NP),
            "sel3": sel3.astype(BF16_NP),
            "bmask": bmask.reshape(128, 16 * 128).astype(BF16_NP),
        }
        in_maps.append(m)
    return in_maps, False


def run_cores(in_maps, fold_w=False, **kw):
    nc = build_program(fold_w)
    return run_bass_kernel_spmd(nc, in_maps, list(range(N_CORES)), **kw)


def kernel(**inputs):
    mask = np.asarray(inputs["mask"])
    assert mask.all(), "kernel specialized for all-true mask (spec fill=ones)"
    in_maps, fold_w = prep_core_inputs(**inputs)
    res = run_cores(in_maps, fold_w).results
    out = np.empty((B, S, D), np.float32)
    for core in range(N_CORES):
        b, g = divmod(core, 4)
        o = res[core]["out"].astype(np.float32)
        for jb in range(4):
            out[b, 512 * jb + 128 * g:512 * jb + 128 * (g + 1), :] = \
                o[128 * jb:128 * (jb + 1)]
    return out



# revision 31
# speedup vs baseline: 1.3604x; 1.3604x over previous
"""Trainium2 Bass kernel for nn_Attention_1056561955116 (sparse chunk attention).

Contract: kernel(**inputs) takes FULL unsharded numpy inputs (as produced by
the problem's setup_inputs) and returns the FULL [2, 2048, 1024] f32 output.

Sharding: 8 NeuronCores = 2 batches x 4 head-groups (4 heads / 256 inner dims
each). Per core: QKV projections (bf16, f32 accum) pipelined per 512-token
block with qk-RMS-norm + rotary on vector/scalar engines, block-sparse
attention (41 of 256 chunk pairs) with two heads packed per PE pass,
softmax denominator via the ones-column trick + fast reciprocal. Out
projection is row-parallel (Wo sharded over the core's 256 inner dims):
each core emits a full-[S, D] partial product; the host sums the four
per-batch partials and adds the bias (the unshard step for row-parallel
TP). No device collectives.
"""

import os
import sys

import numpy as np

for _p in ("/opt/trn_rl_repo", "/root/.axon_site/_ro/trn_rl_repo"):
    if os.path.isdir(_p) and _p not in sys.path:
        sys.path.append(_p)

import concourse.bass as bass  # noqa: E402,F401
import concourse.mybir as mybir  # noqa: E402
import concourse.tile as tile  # noqa: E402
from concourse import bacc  # noqa: E402
from concourse.bass_utils import run_bass_kernel_spmd  # noqa: E402

try:
    import ml_dtypes

    BF16_NP = ml_dtypes.bfloat16
except ImportError:  # pragma: no cover
    BF16_NP = np.float32

F32 = mybir.dt.float32
F32R = mybir.dt.float32r
BF16 = mybir.dt.bfloat16
I32 = mybir.dt.int32
AF = mybir.ActivationFunctionType
OP = mybir.AluOpType

B = 2
S = 2048
D = 1024
HEADS = 16
HD = 64
CHUNK = 128
N_CORES = 8
HG = 4          # heads per core
E = HG * HD     # 256 inner dims per core
NCH = S // CHUNK  # 16 chunks
EPS = 1e-6
SCALE = 1.0 / float(HD) ** 0.5


def _build_chunk_lists():
    n_cache = S // (2 * CHUNK)  # 8
    max_lookback = 5
    K = []  # K[j] = key chunks visible to query chunk j, self last
    for j in range(NCH):
        ks = []
        rel = j - n_cache
        if rel >= 0:
            ks = [c for c in range(n_cache)
                  if c < rel and c >= rel - max_lookback]
        ks.append(j)
        K.append(ks)
    return K


K_J = _build_chunk_lists()
assert sum(len(k) for k in K_J) == 41

# Attention plan per query bank-block jb: groups of <= 4 (c, j) chunk pairs
# (512 logit columns per head = 1 psum bank per head). Entries within a
# group are c-major merged j-runs with uniform start/stop flags; c-ascending
# order guarantees each j's start entry (first cache chunk) executes first
# and its stop entry (self chunk) executes last.


def _build_attn_plan():
    plans = []  # plans[jb] = list of (entries, ncol); entry=(c,j0,jn,col,st,sp)
    for jb in range(4):
        js = range(4 * jb, 4 * jb + 4)
        cs = sorted({c for j in js for c in K_J[j]})
        seq = [(c, j) for c in cs for j in js if c in K_J[j]]
        groups = [seq[i:i + 4] for i in range(0, len(seq), 4)]
        gplans = []
        for grp in groups:
            entries = []
            col = 0
            for (c, j) in grp:
                st = (c == K_J[j][0])
                sp = (c == j)
                if (entries and entries[-1][0] == c
                        and entries[-1][1] + entries[-1][2] == j
                        and entries[-1][4] == st and entries[-1][5] == sp):
                    e = entries[-1]
                    entries[-1] = (e[0], e[1], e[2] + 1, e[3], e[4], e[5])
                else:
                    entries.append((c, j, 1, col, st, sp))
                col += CHUNK
            gplans.append((entries, col))
        plans.append(gplans)
    return plans


ATTN_PLAN = _build_attn_plan()


def _dev2orig():
    # rotation pair i -> device lanes (32*(i//16) + i%16, +16): the rope
    # swap partner is lane ^ 16 inside each 32-lane group.
    d2o = np.zeros(HD, dtype=np.int64)
    for a in range(HD):
        q32, r0 = divmod(a, 32)
        o, r = divmod(r0, 16)
        d2o[a] = 2 * (16 * q32 + r) + o
    return d2o


DEV2ORIG = _dev2orig()
SWAP = DEV2ORIG[np.arange(HD) ^ 16]

_PROGRAM_CACHE = {}


class _Bacc(bacc.Bacc):
    def insert_act_table_loads(self):
        from concourse.hw_specs import get_activation_tables

        has_activation = any(
            isinstance(i, mybir.InstActivation)
            for b in self.main_func.blocks
            for i in b.instructions
        )
        if not has_activation:
            return
        A = mybir.ActivationFunctionType
        keep_only_in = "natural_log_exp_and_others"
        steer = {A.Exp, A.Ln}
        tables = []
        for name, fns in get_activation_tables(self.m.arch).items():
            if name != keep_only_in:
                fns = {f for f in fns if f not in steer}
            tables.append((name, fns))
        import bass_rust as _bass_rust
        _bass_rust.insert_act_table_loads(self, tables)


def build_program(fold_w=False):
    key = "prog"
    if key in _PROGRAM_CACHE:
        return _PROGRAM_CACHE[key]

    nc = _Bacc("TRN2", target_bir_lowering=False, debug=False,
               num_devices=N_CORES)

    xT = nc.dram_tensor("xT", [D, S], BF16, kind="ExternalInput")
    wqT = nc.dram_tensor("wqT", [D, E], BF16, kind="ExternalInput")
    wkT = nc.dram_tensor("wkT", [D, E], BF16, kind="ExternalInput")
    wvT = nc.dram_tensor("wvT", [D, E], BF16, kind="ExternalInput")
    woT = nc.dram_tensor("woT", [128, 2, D], BF16, kind="ExternalInput")
    bqd = nc.dram_tensor("bq", [E], F32, kind="ExternalInput")
    bkd = nc.dram_tensor("bk", [E], F32, kind="ExternalInput")
    cosqd = nc.dram_tensor("cosq", [128, S], BF16, kind="ExternalInput")
    sinqd = nc.dram_tensor("sinq", [128, S], BF16, kind="ExternalInput")
    coskd = nc.dram_tensor("cosk", [128, S], BF16, kind="ExternalInput")
    sinkd = nc.dram_tensor("sink", [128, S], BF16, kind="ExternalInput")
    ind4d = nc.dram_tensor("ind4", [128, 4, 8], BF16, kind="ExternalInput")
    sel3d = nc.dram_tensor("sel3", [8, 512], BF16, kind="ExternalInput")

    out_ext = nc.dram_tensor("out", [S, D], BF16, kind="ExternalOutput")
    kdbg = bool(os.environ.get("KDBG"))
    if kdbg:
        dbg = {
            "dbg_qh0": nc.dram_tensor("dbg_qh0", [128, S], BF16, kind="ExternalOutput"),
            "dbg_kh0": nc.dram_tensor("dbg_kh0", [128, S], BF16, kind="ExternalOutput"),
            "dbg_rallb": nc.dram_tensor("dbg_rallb", [8, S], BF16, kind="ExternalOutput"),
            "dbg_pch00": nc.dram_tensor("dbg_pch00", [128, 1024], BF16, kind="ExternalOutput"),
            "dbg_usl00": nc.dram_tensor("dbg_usl00", [1, 512], F32, kind="ExternalOutput"),
            "dbg_oun00": nc.dram_tensor("dbg_oun00", [HD, 512], BF16, kind="ExternalOutput"),
            "dbg_ofin0": nc.dram_tensor("dbg_ofin0", [128, S], BF16, kind="ExternalOutput"),
        }

    with tile.TileContext(nc) as tc:
        with tc.tile_pool(name="persist", bufs=1) as pp, \
             tc.tile_pool(name="tmp", bufs=1) as tp:

            x_sb = pp.tile([128, 8, S], BF16, name="x_sb")
            wq_sb = pp.tile([128, 8, E], BF16, name="wq_sb")
            wk_sb = pp.tile([128, 8, E], BF16, name="wk_sb")
            wv_sb = pp.tile([128, 8, E], BF16, name="wv_sb")
            wo_sb = pp.tile([128, 2, D], BF16, name="wo_sb")
            bq_sb = pp.tile([128, 2], F32, name="bq_sb")
            bk_sb = pp.tile([128, 2], F32, name="bk_sb")
            ind4_sb = pp.tile([128, 4, 8], BF16, name="ind4_sb")
            sel3_sb = pp.tile([8, 512], BF16, name="sel3_sb")
            on64f_sb = pp.tile([128, HD], F32, name="on64f_sb")
            on64_sb = pp.tile([128, HD], F32R, name="on64_sb")
            eps_sb = pp.tile([128, 1], F32, name="eps_sb")

            qt_sb = [pp.tile([128, S], BF16, name=f"qt{t}") for t in range(2)]
            kt_sb = [pp.tile([128, S], BF16, name=f"kt{t}") for t in range(2)]
            qh_sb = [pp.tile([128, S], BF16, name=f"qh{t}") for t in range(2)]
            kh_sb = [pp.tile([128, S], BF16, name=f"kh{t}") for t in range(2)]
            v_sb = pp.tile([128, NCH, HG * (HD + 1)], BF16, name="v_sb")
            o_fin = [pp.tile([128, S], BF16, name=f"ofin{t}") for t in range(2)]
            rallb = pp.tile([8, S], BF16, name="rallb")
            cosq = pp.tile([128, S], BF16, name="cosq")
            sinq = pp.tile([128, S], BF16, name="sinq")
            cosk = pp.tile([128, S], BF16, name="cosk")
            sink = pp.tile([128, S], BF16, name="sink")

            # ---- input DMAs: x m0 + q weights first so the PE starts ASAP;
            # everything else ordered by first use ----
            for dc in range(8):
                nc.sync.dma_start(x_sb[:, dc, 0:512], xT[128 * dc:128 * (dc + 1), 0:512])
            nc.sync.dma_start(wq_sb[:], wqT.ap().rearrange("(c p) e -> p c e", p=128))
            nc.sync.dma_start(bq_sb[:], bqd.ap().rearrange("(t p) -> p t", p=128))
            nc.sync.dma_start(wk_sb[:], wkT.ap().rearrange("(c p) e -> p c e", p=128))
            nc.sync.dma_start(bk_sb[:], bkd.ap().rearrange("(t p) -> p t", p=128))
            nc.sync.dma_start(ind4_sb[:], ind4d[:])
            nc.sync.dma_start(sel3_sb[:], sel3d[:])
            nc.sync.dma_start(cosq[:], cosqd[:])
            nc.sync.dma_start(sinq[:], sinqd[:])
            nc.sync.dma_start(cosk[:], coskd[:])
            nc.sync.dma_start(sink[:], sinkd[:])
            nc.sync.dma_start(wv_sb[:], wvT.ap().rearrange("(c p) e -> p c e", p=128))
            for m in range(1, 4):
                for dc in range(8):
                    nc.sync.dma_start(
                        x_sb[:, dc, 512 * m:512 * (m + 1)],
                        xT[128 * dc:128 * (dc + 1), 512 * m:512 * (m + 1)])
            nc.sync.dma_start(wo_sb[:], woT[:])

            nc.vector.memset(eps_sb[:], EPS)
            nc.vector.memset(on64f_sb[:], 1.0)
            nc.vector.tensor_copy(on64_sb[:], on64f_sb[:])
            nc.vector.memset(
                v_sb[:].rearrange("p t (h x) -> p t h x", x=HD + 1)[:, :, :, HD:],
                1.0)

            # tiles: i -> (weights, bias, pre-rope dst, rope tables, rope dst)
            tiles = [
                (wq_sb, bq_sb, qt_sb[0], 0, cosq, sinq, qh_sb[0]),
                (wq_sb, bq_sb, qt_sb[1], 1, cosq, sinq, qh_sb[1]),
                (wk_sb, bk_sb, kt_sb[0], 0, cosk, sink, kh_sb[0]),
                (wk_sb, bk_sb, kt_sb[1], 1, cosk, sink, kh_sb[1]),
            ]

            # ---------------- m-loop: projections + rms + rope ----------------
            with tc.tile_pool(name="p1", bufs=1, space="PSUM") as p1:
                for m in range(4):
                    msl = slice(512 * m, 512 * (m + 1))
                    # q0 q1 k0 k1 projections; each tile's pssq matmul is
                    # staggered one tile later so the PE never waits on the
                    # scalar copy + vector square feeding it
                    sqs = []
                    for i, (w_sb, b_sb, dst, t, ct, st_t, rdst) in enumerate(tiles):
                        ps = p1.tile([128, 512], F32, tag="proj", bufs=2)
                        for dc in range(8):
                            nc.tensor.matmul(
                                ps[:], w_sb[:, dc, 128 * t:128 * (t + 1)],
                                x_sb[:, dc, msl],
                                start=(dc == 0), stop=(dc == 7))
                        nc.scalar.activation(dst[:, msl], ps[:], AF.Identity,
                                             bias=b_sb[:, t:t + 1])
                        sq = tp.tile([128, 512], BF16, tag="sq", bufs=3)
                        nc.vector.tensor_tensor(sq[:], dst[:, msl], dst[:, msl],
                                                OP.mult)
                        sqs.append(sq)
                        if i >= 1:
                            if i == 1:
                                pssq = p1.tile([128, 512], F32, tag="pssq",
                                               bufs=2)
                            nc.tensor.matmul(pssq[0:8, :], ind4_sb[:, i - 1, :],
                                             sqs[i - 1][:],
                                             start=(i == 1), stop=False)
                    # v projection for this m-block's 4 chunks; last pssq
                    # rides after the first v chunk
                    for k, tn in enumerate(range(4 * m, 4 * m + 4)):
                        vps = p1.tile([128, 256], F32, tag="vps", bufs=2)
                        for dc in range(8):
                            nc.tensor.matmul(
                                vps[:], x_sb[:, dc, 128 * tn:128 * (tn + 1)],
                                wv_sb[:, dc, :], start=(dc == 0), stop=(dc == 7))
                        vdst = v_sb[:, tn].rearrange("p (h x) -> p h x",
                                                     x=HD + 1)[:, :, :HD]
                        nc.scalar.activation(
                            vdst, vps[:].rearrange("p (h d) -> p h d", d=HD),
                            AF.Copy)
                        if k == 0:
                            nc.tensor.matmul(pssq[0:8, :], ind4_sb[:, 3, :],
                                             sqs[3][:], start=False, stop=True)
                    # rsqrt factors for this m-block (rows 2*i + [0,1])
                    rall_m = tp.tile([8, 512], F32, tag="rall", bufs=2)
                    nc.scalar.activation(rall_m[:], pssq[0:8, :], AF.Ln,
                                         scale=1.0 / HD, bias=eps_sb[0:8, 0:1])
                    nc.scalar.activation(rallb[:, msl], rall_m[:], AF.Exp,
                                         scale=-0.5)
                    # rope + rms scale, on vector engine; rb broadcast on PE
                    for i, (w_sb, b_sb, src, t, ct, st_t, rdst) in enumerate(tiles):
                        sh = tp.tile([128, 512], BF16, tag="sh", bufs=2)
                        nc.vector.stream_shuffle(sh[:].bitcast(I32),
                                                 src[:, msl].bitcast(I32),
                                                 [l ^ 16 for l in range(32)])
                        t1 = tp.tile([128, 512], BF16, tag="t1", bufs=2)
                        nc.vector.tensor_tensor(t1[:], src[:, msl], ct[:, msl],
                                                OP.mult)
                        t2 = tp.tile([128, 512], BF16, tag="t2", bufs=2)
                        nc.vector.tensor_tensor(t2[:], sh[:], st_t[:, msl],
                                                OP.mult)
                        nc.vector.tensor_tensor(t1[:], t1[:], t2[:], OP.add)
                        rb = p1.tile([128, 512], F32, tag="rb", bufs=2)
                        nc.tensor.matmul(rb[:], sel3_sb[:, 128 * i:128 * (i + 1)],
                                         rallb[:, msl], start=True, stop=True)
                        nc.vector.tensor_tensor(rdst[:, msl], t1[:], rb[:],
                                                OP.mult)

            if kdbg:
                nc.sync.dma_start(dbg["dbg_qh0"][:], qh_sb[0][:])
                nc.sync.dma_start(dbg["dbg_kh0"][:], kh_sb[0][:])
                nc.sync.dma_start(dbg["dbg_rallb"][:], rallb[:])

            # ---------------- attention + row-parallel out-proj ----------------
            def emit_attn(jb, p2):
                n_ent = sum(len(e) for e, _ in ATTN_PLAN[jb])
                for th in range(2):
                    pvs = []
                    for h in range(2):
                        pv = p2.tile([HD + 1, 512], F32, tag="pv", bufs=2)
                        pvs.append(pv)
                    ei = 0
                    for (entries, ncol) in ATTN_PLAN[jb]:
                        lt = p2.tile([128, 1024], F32, tag="lt", bufs=2)
                        for (c, j0, jn, col, st, sp) in entries:
                            csl = slice(128 * c, 128 * (c + 1))
                            jsl = slice(128 * j0, 128 * (j0 + jn))
                            nc.tensor.matmul(
                                lt[:, col:col + 128 * jn],
                                kh_sb[th][0:64, csl], qh_sb[th][0:64, jsl],
                                start=True, stop=True)
                            nc.tensor.matmul(
                                lt[:, 512 + col:512 + col + 128 * jn],
                                kh_sb[th][64:128, csl], qh_sb[th][64:128, jsl],
                                start=True, stop=True)
                        pch = tp.tile([128, 1024], BF16, tag="pch", bufs=3)
                        if ncol == 512:
                            nc.scalar.activation(pch[:, 0:1024], lt[:, 0:1024],
                                                 AF.Exp, scale=SCALE)
                        else:
                            for h in range(2):
                                nc.scalar.activation(
                                    pch[:, 512 * h:512 * h + ncol],
                                    lt[:, 512 * h:512 * h + ncol],
                                    AF.Exp, scale=SCALE)
                        if kdbg and jb == 0 and th == 0 and ei == 0:
                            nc.sync.dma_start(dbg["dbg_pch00"][:], pch[:])
                        for (c, j0, jn, col, st, sp) in entries:
                            jj = j0 - 4 * jb
                            for h in range(2):
                                vsl = v_sb[:, c].rearrange(
                                    "p (h x) -> p h x", x=HD + 1)[:, 2 * th + h, :]
                                nc.tensor.matmul(
                                    pvs[h][:, 128 * jj:128 * (jj + jn)], vsl,
                                    pch[:, 512 * h + col:512 * h + col + 128 * jn],
                                    start=(ei == 0), stop=(ei == n_ent - 1))
                            ei += 1
                    # normalize: o = pv[:64] / den; 1/den via DVE fast
                    # reciprocal, f32r-rounded on the idle Pool engine,
                    # broadcast to 64 lanes by a full-rate f32r matmul,
                    # downcast to sbuf bf16 on scalar for the legal mult
                    for h in range(2):
                        pv = pvs[h]
                        usl = tp.tile([HD + 1, 512], F32, tag="usl", bufs=2)
                        nc.scalar.activation(usl[HD:HD + 1, :],
                                             pv[HD:HD + 1, :], AF.Ln)
                        uslr = tp.tile([HD + 1, 512], F32R, tag="uslr", bufs=2)
                        nc.vector.tensor_copy(uslr[HD:HD + 1, :],
                                              usl[HD:HD + 1, :])
                        rdb = p2.tile([128, 512], F32, tag="ops", bufs=2)
                        nc.tensor.matmul(
                            rdb[0:HD, :], on64_sb[HD:HD + 1, 0:HD],
                            uslr[HD:HD + 1, :],
                            start=True, stop=True)
                        rdb_sb = tp.tile([HD, 512], BF16, tag="rdb_sb", bufs=2)
                        nc.scalar.activation(rdb_sb[:], rdb[0:HD, :], AF.Exp,
                                             scale=-1.0)
                        nc.vector.tensor_tensor(
                            o_fin[th][64 * h:64 * (h + 1),
                                      512 * jb:512 * (jb + 1)],
                            pv[0:HD, :], rdb_sb[:], OP.mult)
                        if kdbg and jb == 0 and th == 0 and h == 0:
                            nc.sync.dma_start(dbg["dbg_usl00"][:],
                                              usl[HD:HD + 1, :])

            def emit_oproj(jb, p2):
                # partial out-proj for this jb's 512 tokens: contract over
                # this core's 256 inner dims (o_fin rows are already o^T);
                # psum -> bf16 downcast split across scalar + vector
                for tc_ in range(4):
                    tok = 512 * jb + 128 * tc_
                    st = tp.tile([128, 1024], BF16, tag="st", bufs=3)
                    for dh in range(2):
                        ops = p2.tile([128, 512], F32, tag="ops", bufs=2)
                        for th in range(2):
                            nc.tensor.matmul(
                                ops[:], o_fin[th][:, tok:tok + 128],
                                wo_sb[:, th, 512 * dh:512 * (dh + 1)],
                                start=(th == 0), stop=(th == 1))
                        if dh == 0:
                            nc.scalar.activation(st[:, 0:512], ops[:], AF.Copy)
                        else:
                            nc.vector.tensor_copy(st[:, 512:1024], ops[:])
                    nc.sync.dma_start(out_ext[tok:tok + 128, :], st[:])

            # interleave: oproj jb rides behind attn jb+1 so the PE queue
            # never stalls on the normalize chain of the block it just made
            with tc.tile_pool(name="p2", bufs=1, space="PSUM") as p2:
                emit_attn(0, p2)
                emit_attn(1, p2)
                emit_oproj(0, p2)
                emit_attn(2, p2)
                emit_oproj(1, p2)
                emit_attn(3, p2)
                emit_oproj(2, p2)
                emit_oproj(3, p2)
            if kdbg:
                nc.sync.dma_start(dbg["dbg_ofin0"][:, 0:512], o_fin[0][:, 0:512])

    nc.compile()
    _PROGRAM_CACHE[key] = nc
    return nc


def prep_core_inputs(x, mask, freqs, Wq, bq, Wk, bk, Wv, bv, Wo, bo, qw, kw):
    qw = np.asarray(qw, np.float64)
    kw = np.asarray(kw, np.float64)

    perm = (np.arange(HEADS)[:, None] * HD + DEV2ORIG[None, :]).reshape(-1)
    Wq_p = np.asarray(Wq, np.float32)[perm]
    Wk_p = np.asarray(Wk, np.float32)[perm]
    bq_p = np.asarray(bq, np.float32)[perm]
    bk_p = np.asarray(bk, np.float32)[perm]

    fr = np.asarray(freqs, np.float64)[:, DEV2ORIG].T  # [64, S] dev-lane order
    sign = np.where((np.arange(HD) % 32) < 16, -1.0, 1.0)
    cos_d = np.cos(fr)
    sin_d = np.sin(fr) * sign[:, None]
    cosq1 = cos_d * qw[DEV2ORIG][:, None]
    sinq1 = sin_d * qw[SWAP][:, None]
    cosk1 = cos_d * kw[DEV2ORIG][:, None]
    sink1 = sin_d * kw[SWAP][:, None]

    def dup(a):
        return np.concatenate([a, a], axis=0).astype(BF16_NP)

    ind4 = np.zeros((128, 4, 8), np.float32)
    for i in range(4):
        ind4[0:64, i, 2 * i] = 1.0
        ind4[64:128, i, 2 * i + 1] = 1.0
    sel3 = np.zeros((8, 512), np.float32)
    for i in range(4):
        for b2 in range(2):
            sel3[2 * i + b2, 128 * i + 64 * b2:128 * i + 64 * (b2 + 1)] = 1.0

    x = np.asarray(x, np.float32)
    Wo = np.asarray(Wo, np.float32)
    Wv = np.asarray(Wv, np.float32)
    bv = np.asarray(bv, np.float32)
    bo = np.asarray(bo, np.float32)
    woT = np.ascontiguousarray(Wo.T)  # [E_full=1024, D]
    bo_eff = bv @ woT + bo            # [D], added on host

    in_maps = []
    for core in range(N_CORES):
        b, g = divmod(core, 4)
        esl = slice(E * g, E * (g + 1))
        # this core's Wo rows, laid out [128 lanes, th, D] to match o_fin
        wo_c = woT[E * g:E * (g + 1)].reshape(2, 128, D).transpose(1, 0, 2)
        m = {
            "xT": np.ascontiguousarray(x[b].T).astype(BF16_NP),
            "wqT": np.ascontiguousarray(Wq_p[esl].T).astype(BF16_NP),
            "wkT": np.ascontiguousarray(Wk_p[esl].T).astype(BF16_NP),
            "wvT": np.ascontiguousarray(Wv[esl].T).astype(BF16_NP),
            "woT": np.ascontiguousarray(wo_c).astype(BF16_NP),
            "bq": bq_p[esl].copy(),
            "bk": bk_p[esl].copy(),
            "cosq": dup(cosq1),
            "sinq": dup(sinq1),
            "cosk": dup(cosk1),
            "sink": dup(sink1),
            "ind4": ind4.astype(BF16_NP),
            "sel3": sel3.astype(BF16_NP),
        }
        in_maps.append(m)
    return in_maps, bo_eff


def run_cores(in_maps, fold_w=False, **kw):
    nc = build_program(fold_w)
    return run_bass_kernel_spmd(nc, in_maps, list(range(N_CORES)), **kw)


def combine_outputs(results, bo_eff):
    out = np.empty((B, S, D), np.float32)
    for b in range(B):
        acc = np.zeros((S, D), np.float32)
        for g in range(4):
            acc += np.asarray(results[4 * b + g]["out"], np.float32)
        out[b] = acc + bo_eff[None, :]
    return out


def kernel(**inputs):
    mask = np.asarray(inputs["mask"])
    assert mask.all(), "kernel specialized for all-true mask (spec fill=ones)"
    in_maps, bo_eff = prep_core_inputs(**inputs)
    res = run_cores(in_maps).results
    return combine_outputs(res, bo_eff)
